# revision 50
# baseline (speedup 1.0000x reference)
"""Binarized conv1d (k=7, pad=3 with -1.0) + maxpool(2) + PReLU + BatchNorm1d
(training stats) fused Trainium2 kernel, data-parallel over batch N across 8
NeuronCores with an on-chip AllReduce for the BN batch statistics.

Contract: kernel(**inputs) takes the FULL inputs from setup_inputs() and
returns the FULL [128, 128, 2048] float32 output.

Changes over the 218us baseline:
  - ramp: pair-0's x load split into 3 column chunks on 3 different HWDGE
    queues (vector/tensor/sync) so they land in parallel; pair-0's at-copies
    ride the scalar queue so they never sit behind pair-1's 2.1MB x load;
    sign/copies/matmuls emitted per chunk (first matmul ~9us instead of 31us).
  - steady: per batch, half0 drains via the B scheme (ACT PReLU straight
    off PSUM — prelu is monotone — then a 2-port DVE f16 strided max with
    the sum(y) accum) and half1 entirely on DVE (tensor_reduce max off PSUM,
    then prelu as (m*alpha) max m, valid for 0<alpha<1). sum(y^2) runs
    mostly on ACT (Square+accum). ACT and DVE both land ~10.2us/pair; the
    PE (~8.7us/pair) keeps up. (The gpsimd/Q7 engine measured ~15ns/col on
    tensor ops — useless for any of this, it only keeps the pad memsets.)
  - output written as [128, NB*2048] f16 (contiguous 8KB/partition runs per
    pair store = half the store descriptors); host permutes to [NB,CO,LO].
  - pass-2 normalize split across DVE (tensor_scalar) and ACT (Identity with
    per-channel scale/bias APs); stores spread across all 4 HWDGE queues.
"""

import uuid

import numpy as np
import ml_dtypes
import jax
import bass_rust as _br

# The jax persistent compilation cache mis-keys bass_exec custom-call
# executables (the embedded NEFF differs while the cache key does not),
# which can hand back a stale executable and wedge the device. Disable it.
jax.config.update("jax_enable_compilation_cache", False)

import concourse.bacc as bacc
import concourse.mybir as mybir
import concourse.tile as tile
from concourse.bass_utils import run_bass_kernel_spmd

AF = mybir.ActivationFunctionType
ALU = mybir.AluOpType

N_CORES = 8
N = 128            # total batch
NB = N // N_CORES  # batches per core = 16
CI = 64            # in channels
CO = 128           # out channels
L = 4096           # input length
LP = L + 7         # padded width: cols 0-2 pad, 3..4098 data, 4099-4102 pad
LO = L // 2        # pooled output length = 2048
K = 7              # kernel taps
PAD_VAL = -1.0
EPS = 1e-5
M_GLOBAL = float(N * LO)  # BN reduction count per channel

XT_BUFS = 4
AT_BUFS = 4
# pair-0 ramp column chunk boundaries (sign/copy/matmul pipeline): chunk A
# is small so the first sign+copies+matmuls fire as early as possible
# (the whole device's HBM is saturated during the ramp — all 8 cores pull
# their pair-0 concurrently — so the first 0.13MB is what we can get early).
# A covers the t0 matmuls, B covers t1-t3, C covers half 1.
C0 = 520
C1 = 2056
# steady-state sign/copy split: halves the ACT head-of-line blocking and
# lets each pair's first matmuls start after only half the copies
SPLIT = 2056

# Engine load-balance: ACT carries sign (3.6us/pair) + 2 B-half prelus
# (1.85 each) + most of the sumsq; DVE carries 2 B-half pools (1.14) + 2
# DVE-only half drains (2.2 + 1.15) + the sumsq remainder.
SQ_ACT = 1280  # cols [LO-SQ_ACT, LO) of each batch's sumsq on ACT


def _build(alpha: float):
    nc = bacc.Bacc("TRN2", target_bir_lowering=False, debug=False,
                   num_devices=N_CORES)

    # x ships as bf16: the kernel only consumes sign(x), which the cast
    # preserves; this halves the host->device bytes, the HBM read traffic,
    # and the SBUF x-tile footprint
    xs = nc.dram_tensor("xs", [NB * CI, L], mybir.dt.bfloat16, kind="ExternalInput")
    wts = nc.dram_tensor("wts", [128, 8 * 128], mybir.dt.float8e4, kind="ExternalInput")
    gb = nc.dram_tensor("gb", [128, 2], mybir.dt.float32, kind="ExternalInput")
    out = nc.dram_tensor("out", [128, NB * LO], mybir.dt.float16, kind="ExternalOutput")

    with tile.TileContext(nc) as tc:
        with (
            tc.tile_pool(name="wp", bufs=1) as wp,
            tc.tile_pool(name="xp", bufs=XT_BUFS) as xp,
            tc.tile_pool(name="ap", bufs=AT_BUFS) as ap_pool,
            tc.tile_pool(name="pp", bufs=2, space="PSUM") as pp,
            tc.tile_pool(name="mp", bufs=4) as mp,
            tc.tile_pool(name="yp", bufs=NB) as yp,
            tc.tile_pool(name="sp", bufs=1) as sp,
            tc.tile_pool(name="qp", bufs=2) as qp,
            tc.tile_pool(name="op", bufs=4) as op_pool,
        ):
            # weights/params go via the scalar-engine HWDGE queue so the
            # ramp x chunks own the sync/vector/tensor queues from t=0
            wt = wp.tile([128, 8 * 128], mybir.dt.float8e4)
            nc.scalar.dma_start(wt[:], wts[:])
            gbt = wp.tile([128, 2], mybir.dt.float32)
            nc.scalar.dma_start(gbt[:], gb[:])

            # trigger the ACT table load during the DMA ramp, off-path
            # (the Sqrt table would otherwise load lazily right inside the
            # stats->all-reduce critical tail, costing ~1.3us on every core)
            warm = wp.tile([128, 1], mybir.dt.float32)
            nc.vector.memset(warm[:], 1.0)
            nc.scalar.activation(warm[:], warm[:], AF.Sign)
            nc.scalar.activation(warm[:], warm[:], AF.Sqrt)

            _build_pass(nc, tc, xs, out, wt, gbt, alpha,
                        xp, ap_pool, pp, mp, yp, sp, qp, op_pool)

    nc.compile()
    nc.m.name = f"bk{uuid.uuid4().hex[:10]}"
    return nc


def _build_pass(nc, tc, xs, out, wt, gbt, alpha,
                xp, ap_pool, pp, mp, yp, sp, qp, op_pool):
    # stats: cols 0:32 per-half-tile sum accums, 32:48 per-batch sumsq (DVE
    # half), 48:64 per-batch sumsq (ACT half)
    stats = sp.tile([128, 64], mybir.dt.float32, name="stats", tag="stats")

    # all-reduce buffers (persistent pool: stable addresses, single-writer
    # slots, so remote writes may land any time after the barrier)
    xbuf = sp.tile([128, 16], mybir.dt.float32, name="xbuf", tag="xbuf")
    g = sp.tile([128, 2], mybir.dt.float32, name="g", tag="g")
    rsem = nc.alloc_semaphore("ar_remote")
    lsem = nc.alloc_semaphore("ar_local")

    def _ar_descgen():
        # Each core broadcasts its [128,2] partial (sum, sumsq) into peer
        # SBUFs: for delta in 1..7, core c sends to core c^delta, landing in
        # xbuf slot delta; the XOR pairing makes every slot single-writer.
        # Descriptor generation runs here, during pass 1 while gpsimd idles;
        # the descriptors' deferred source read means only trigger_dma + the
        # 1KB transfers sit on the post-stats critical path.
        for delta in range(1, 8):
            rd = [None] * 8
            rd[delta] = (0, delta)
            nc.gpsimd.remote_dma_broadcast(
                xbuf[:, 2 * delta:2 * delta + 2], xbuf[:, 0:2],
                rsem, lsem, rdests=rd,
            )

    def _dr_rhs(at, col):
        # DoubleRow moving operand: [128, 2 k-tiles, 512] where the 2
        # contraction elements per cell are the activation at col+{0,+2}
        # (adjacent tap-pair planes = column shifts in the same at buffer).
        rhs = at[:, col:col + 512].copy()
        part = rhs.ap[0]
        rhs.ap = _br.VecI64Pair([(part[0], part[1]), (2, 2), (1, 512)])
        return rhs

    def _xt_load(bp):
        xt = xp.tile([128, LP], mybir.dt.bfloat16, name=f"xt{bp}", tag="xt")
        # the x DMA only writes 3:4099; the pad columns of the xt tile are
        # NEVER read — the binarize skips them and the at-tile pads are
        # memset directly in fp8 (see _pair_prep)
        # everything rides the sync HWDGE queue; emission order guarantees
        # each pair's latency-critical at-copies precede the NEXT pair's big
        # x transfer, so the copies (which gate the matmuls) never queue
        # behind a 6us load
        nc.sync.dma_start(xt[:, 3:L + 3], xs[bp * 128:(bp + 1) * 128, :])
        return xt

    def _xt_load0():
        # Pair 0's load is column-split across THREE pool tiles: A and B
        # FIFO on the sync queue (A lands in ~2us, its sign fires early).
        # Chunk C is NOT loaded here: it is emitted inside _prep0_chunk's
        # caller AFTER the A/B at-copies on the scalar queue, so the HWDGE
        # head-of-line blocking prioritizes the latency-critical copies over
        # C's bulk (during the ramp the device HBM is saturated by all 8
        # cores and bulk descriptors starve latency-critical ones).
        xta = xp.tile([128, LP], mybir.dt.bfloat16, name="xt0a", tag="xt")
        xtb = xp.tile([128, LP], mybir.dt.bfloat16, name="xt0b", tag="xt")
        xtc = xp.tile([128, LP], mybir.dt.bfloat16, name="xt0c", tag="xt")
        nc.sync.dma_start(xta[:, 3:C0], xs[0:128, 0:C0 - 3])
        nc.sync.dma_start(xtb[:, C0:C1], xs[0:128, C0 - 3:C1 - 3])
        return xta, xtb, xtc

    def _pair_tiles(bp):
        at0 = ap_pool.tile([128, LP], mybir.dt.float8e4,
                           name=f"at{2 * bp}", tag="at")
        at1 = ap_pool.tile([128, LP], mybir.dt.float8e4,
                           name=f"at{2 * bp + 1}", tag="at")
        return at0, at1

    def _binarize(dst, src):
        # sign(x) in fp8 on ACT (per-lane throughput: [128,*] costs the
        # same as [64,*]). The Q7/gpsimd engine measured ~15ns/col on
        # tensor_scalar — 18x too slow to take any of this work.
        nc.scalar.activation(dst, src, AF.Sign)

    def _pair_prep(bp, xt):
        # Binarize per batch-pair, split in two column chunks so the ACT
        # queue interleaves prelus between the halves and the first-half
        # copies (which gate the next pair's first matmuls) issue ~2us
        # sooner. It lands in the even batch's tile at0: rows 0:64 are b0's
        # direct half already; rows 64:128 transiently hold b1's direct
        # data, which is copied out to at1 (direct + shifted) before being
        # overwritten by b0's shifted copy. b1's matmuls run first.
        at0, at1 = _pair_tiles(bp)
        for c0, c1 in ((0, SPLIT), (SPLIT, LP)):
            # binarize only the data columns; the pad columns are stamped
            # directly in fp8 (sign(-1) = -1) so the xt pads are never read
            if c0 == 0:
                nc.vector.memset(at0[:, 0:3], -1.0)
                _binarize(at0[:, 3:c1], xt[:, 3:c1])
            elif c1 == LP:
                nc.vector.memset(at0[:, L + 3:LP], -1.0)
                _binarize(at0[:, c0:L + 3], xt[:, c0:L + 3])
            else:
                _binarize(at0[:, c0:c1], xt[:, c0:c1])
            nc.sync.dma_start(at1[64:128, c0:c1], at0[64:128, c0:c1])
            lo = max(c0 - 1, 0)
            nc.sync.dma_start(at1[0:64, lo:c1 - 1], at0[64:128, lo + 1:c1])
            nc.sync.dma_start(at0[64:128, lo:c1 - 1], at0[0:64, lo + 1:c1])
        return at0, at1

    def _prep0_chunk(at0, at1, xt, c0, c1, eng):
        # ramp: binarize+copies+shift for columns [c0, c1) only, tracking
        # the chunked x load. The copies are latency-critical (they gate the
        # matmuls) and tiny, while the device HBM is saturated by every
        # core's ramp; each chunk's copies sit at the HEAD of a HWDGE queue
        # (eng) so the queue's head-of-line blocking holds the bulk streams
        # queued behind them until the copies issue. Pad columns are stamped
        # in fp8, never read from xt.
        if c0 == 0:
            nc.vector.memset(at0[:, 0:3], -1.0)
            _binarize(at0[:, 3:c1], xt[:, 3:c1])
        elif c1 == LP:
            nc.vector.memset(at0[:, L + 3:LP], -1.0)
            _binarize(at0[:, c0:L + 3], xt[:, c0:L + 3])
        else:
            _binarize(at0[:, c0:c1], xt[:, c0:c1])
        eng.dma_start(at1[64:128, c0:c1], at0[64:128, c0:c1])
        lo = max(c0 - 1, 0)
        eng.dma_start(at1[0:64, lo:c1 - 1], at0[64:128, lo + 1:c1])
        eng.dma_start(at0[64:128, lo:c1 - 1], at0[0:64, lo + 1:c1])

    y_tiles = [None] * NB
    pending_sq = []

    def _emit_sq():
        # sum(y^2) split across both fast engines (the gpsimd/Pool engine
        # cannot run TensorScalarPtr/Square on real ISA): DVE takes the first
        # chunk (written by the earlier half0 drain, so it can start during
        # half1's ACT prelu), ACT Square-accum takes the rest.
        b, yt = pending_sq.pop(0)
        sd = LO - SQ_ACT
        sq = qp.tile([128, LO], mybir.dt.bfloat16, name=f"sq{b}", tag="sq")
        nc.scalar.activation(sq[:, sd:LO], yt[:, sd:LO],
                             AF.Square, accum_out=stats[:, 48 + b:49 + b])
        nc.vector.scalar_tensor_tensor(
            sq[:, 0:sd], yt[:, 0:sd], 1.0, yt[:, 0:sd],
            op0=ALU.mult, op1=ALU.mult,
            accum_out=stats[:, 32 + b:33 + b],
        )

    prelu_on_dve = 0.0 < alpha < 1.0

    def _drain_half(b, half, ps, yt):
        hidx = 2 * b + half
        sum_col = stats[:, hidx:hidx + 1]
        dst = yt[:, half * 1024:(half + 1) * 1024]
        if half == 0:
            # B scheme: PReLU straight off PSUM on ACT (prelu is monotone,
            # so prelu(max)=max(prelu)), then a 2-port pooled max on DVE
            # (SBUF f16), accumulating sum(y). Only ONE operand may read
            # PSUM, so this is the cheapest drain: 1.85 ACT + 1.14 DVE.
            pt = mp.tile([128, 2048], mybir.dt.float16,
                         name=f"pt{hidx}", tag="pt")
            nc.scalar.activation(pt[:], ps[:], AF.Prelu, alpha=alpha)
            ptr = pt.rearrange("p (t two) -> p two t", two=2)
            nc.vector.scalar_tensor_tensor(
                dst, ptr[:, 0, :], 1.0, ptr[:, 1, :],
                op0=ALU.mult, op1=ALU.max,
                accum_out=sum_col,
            )
        else:
            # DVE-only drain: 1-port max off PSUM, then prelu as
            # max(alpha*m, m) (valid for 0<alpha<1) with the sum accum
            mt = mp.tile([128, 1024], mybir.dt.float16,
                         name=f"mt{hidx}", tag="mt")
            nc.vector.tensor_reduce(
                mt[:],
                ps.rearrange("p (t two) -> p t two", two=2),
                axis=mybir.AxisListType.X, op=ALU.max,
            )
            if prelu_on_dve:
                nc.vector.scalar_tensor_tensor(
                    dst, mt[:], alpha, mt[:],
                    op0=ALU.mult, op1=ALU.max,
                    accum_out=sum_col,
                )
            else:
                nc.scalar.activation(dst, mt[:], AF.Prelu, alpha=alpha,
                                     accum_out=sum_col)

    # ---- pair 0: chunked ramp ----
    xta, xtb, xtc = _xt_load0()
    at0, at1 = _pair_tiles(0)
    _prep0_chunk(at0, at1, xta, 0, C0, nc.sync)

    yt1 = yp.tile([128, LO], mybir.dt.float16, name="yt1", tag="yt")
    y_tiles[1] = yt1
    yt0 = yp.tile([128, LO], mybir.dt.float16, name="yt0", tag="yt")
    y_tiles[0] = yt0

    def _mm0(ps, ts):
        # b1 matmul group for 512-tiles ts (j0 = 4,6: odd weight layouts)
        for t in ts:
            for pair in range(2):
                j0 = 4 + 2 * pair
                w_pair = wt[:, j0 * 128:(j0 + 2) * 128].rearrange(
                    "p (e c) -> p e c", e=2)
                nc.tensor.matmul(
                    ps[:, (t % 4) * 512:(t % 4 + 1) * 512], w_pair,
                    _dr_rhs(at1, t * 512 + 4 * pair),
                    start=(pair == 0), stop=(pair == 1),
                    perf_mode=mybir.MatmulPerfMode.DoubleRow,
                )

    ps10 = pp.tile([128, 2048], mybir.dt.float32, name="ps1_0", tag="ps")
    _mm0(ps10, (0,))
    _prep0_chunk(at0, at1, xtb, C0, C1, nc.scalar)
    # chunk C's load rides the scalar queue BEHIND the chunk-B copies; the
    # queue's head-of-line blocking keeps its bulk descriptors from
    # starving the copies that gate the matmuls
    nc.scalar.dma_start(xtc[:, C1:L + 3], xs[0:128, C1 - 3:L])
    _mm0(ps10, (1, 2, 3))
    _prep0_chunk(at0, at1, xtc, C1, LP, nc.scalar)
    # pair-1's x load rides the scalar queue at the very END of the ramp
    # chain (behind the chunk-C copies): until every copy has issued, no
    # other bulk stream may touch the DMA engines
    xt_next = xp.tile([128, LP], mybir.dt.bfloat16, name="xt1", tag="xt")
    nc.scalar.dma_start(xt_next[:, 3:L + 3], xs[128:256, :])
    _drain_half(1, 0, ps10, yt1)
    ps11 = pp.tile([128, 2048], mybir.dt.float32, name="ps1_1", tag="ps")
    for pair in range(2):
        j0 = 4 + 2 * pair
        w_pair = wt[:, j0 * 128:(j0 + 2) * 128].rearrange(
            "p (e c) -> p e c", e=2)
        for t in range(4):
            nc.tensor.matmul(
                ps11[:, t * 512:(t + 1) * 512], w_pair,
                _dr_rhs(at1, 2048 + t * 512 + 4 * pair),
                start=(pair == 0), stop=(pair == 1),
                perf_mode=mybir.MatmulPerfMode.DoubleRow,
            )
    # next pair's prep between the two batches (as in steady state)
    prep_next = _pair_prep(1, xt_next)
    _drain_half(1, 1, ps11, yt1)
    # b0: even weight layouts, at0
    for half in range(2):
        h = half * 2048
        ps = pp.tile([128, 2048], mybir.dt.float32, name=f"ps0_{half}", tag="ps")
        for pair in range(2):
            j0 = 2 * pair
            w_pair = wt[:, j0 * 128:(j0 + 2) * 128].rearrange(
                "p (e c) -> p e c", e=2)
            for t in range(4):
                nc.tensor.matmul(
                    ps[:, t * 512:(t + 1) * 512], w_pair,
                    _dr_rhs(at0, h + t * 512 + 4 * pair),
                    start=(pair == 0), stop=(pair == 1),
                    perf_mode=mybir.MatmulPerfMode.DoubleRow,
                )
        _drain_half(0, half, ps, yt0)
    pending_sq.append((1, yt1))
    pending_sq.append((0, yt0))
    while len(pending_sq) > 1:
        _emit_sq()

    # ---- pairs 1..7: steady state ----
    for bp in range(1, NB // 2):
        at0, at1 = prep_next
        # prefetch the next pair's input BEFORE this pair's remaining
        # at-copies enter the sync HWDGE queue
        if bp + 1 < NB // 2:
            xt_next = _xt_load(bp + 1)

        for sub in (1, 0):
            b = 2 * bp + sub
            at = at1 if sub == 1 else at0

            # emit the next pair's sign+copies between this pair's two
            # batches: the ACT engine starts the sign as soon as the x tile
            # lands instead of after this pair's prelus, and the PE never
            # starves waiting for at-tiles
            if sub == 0 and bp + 1 < NB // 2:
                prep_next = _pair_prep(bp + 1, xt_next)

            yt = yp.tile([128, LO], mybir.dt.float16, name=f"yt{b}", tag="yt")
            y_tiles[b] = yt

            for half in range(2):
                h = half * 2048
                ps = pp.tile([128, 2048], mybir.dt.float32,
                             name=f"ps{b}_{half}", tag="ps")
                for pair in range(2):
                    j0 = sub * 4 + 2 * pair
                    w_pair = wt[:, j0 * 128:(j0 + 2) * 128].rearrange(
                        "p (e c) -> p e c", e=2)
                    for t in range(4):
                        nc.tensor.matmul(
                            ps[:, t * 512:(t + 1) * 512],
                            w_pair,
                            _dr_rhs(at, h + t * 512 + 4 * pair),
                            start=(pair == 0), stop=(pair == 1),
                            perf_mode=mybir.MatmulPerfMode.DoubleRow,
                        )
                _drain_half(b, half, ps, yt)

            # defer this batch's sum(y^2) by one batch so it never sits
            # between a PSUM-draining pool and the PE — except in the last
            # pair, where flushing eagerly keeps the final sq off the
            # stats->all-reduce critical tail
            pending_sq.append((b, yt))
            while len(pending_sq) > (0 if bp == NB // 2 - 1 else 1):
                _emit_sq()
        if bp == 2:
            _ar_descgen()
    while pending_sq:
        _emit_sq()

    # ---- local partial stats -> remote-DMA all-reduce -> scale/shift ----
    nc.vector.tensor_reduce(xbuf[:, 0:1], stats[:, 0:32],
                            axis=mybir.AxisListType.X, op=ALU.add)
    nc.vector.tensor_reduce(xbuf[:, 1:2], stats[:, 32:64],
                            axis=mybir.AxisListType.X, op=ALU.add)

    # no_gpsimd_drain: skip the ~45us SWDGE dge_drain at block exit; the
    # kernel-tail drain picks the rings up later, off the critical path.
    with tc.tile_critical(no_gpsimd_drain=True):
        # barrier: no core fires remote writes until every peer has started
        # (prelude AllGather, normally long satisfied by now). The triggers
        # additionally inherit the deferred read of xbuf slot 0, so they wait
        # for the local stats automatically.
        nc.gpsimd.bir_kernel_barrier_wait([list(range(N_CORES))])
        nc.gpsimd.trigger_dma(count=None)
        # 7 arriving broadcasts x (16//8)=2 incs each. Staged waits with
        # marker ops so the trace shows each broadcast's arrival time
        # (distinguishes core skew from descriptor-execution serialization).
        arr = sp.tile([128, 8], mybir.dt.float32, name="arr", tag="arr")
        for k in range(1, 7):
            nc.vector.wait_ge(rsem, 2 * k)
            nc.vector.tensor_scalar(arr[:, k:k + 1], xbuf[:, 0:1], float(k),
                                    None, op0=ALU.mult)
        nc.vector.wait_ge(rsem, 14)
        nc.vector.tensor_reduce(
            g[:, 0:1], xbuf.rearrange("p (s two) -> p two s", two=2)[:, 0:1, :],
            axis=mybir.AxisListType.X, op=ALU.add)
        nc.vector.tensor_reduce(
            g[:, 1:2], xbuf.rearrange("p (s two) -> p two s", two=2)[:, 1:2, :],
            axis=mybir.AxisListType.X, op=ALU.add)

    # mean/var/scale/shift, all [128,1] f32
    v = sp.tile([128, 8], mybir.dt.float32, name="v", tag="v")
    mean, msq_eps, vareps, std, rec, t1, s_col, t_col = (
        v[:, i:i + 1] for i in range(8))
    nc.vector.tensor_scalar(mean, g[:, 0:1], 1.0 / M_GLOBAL, None, op0=ALU.mult)
    # msq_eps = mean^2 - eps
    nc.vector.tensor_scalar(msq_eps, mean, mean, EPS, op0=ALU.mult, op1=ALU.subtract)
    # vareps = ssq/M - (mean^2 - eps) = var + eps
    nc.vector.scalar_tensor_tensor(
        vareps, g[:, 1:2], 1.0 / M_GLOBAL, msq_eps,
        op0=ALU.mult, op1=ALU.subtract)
    nc.scalar.activation(std, vareps, AF.Sqrt)
    # one Newton step: std = 0.5*(std + vareps/std)
    nc.vector.reciprocal(rec, std)
    # t1 = 0.5 * vareps / std
    nc.vector.tensor_scalar(t1, rec, vareps, 0.5, op0=ALU.mult, op1=ALU.mult)
    nc.vector.scalar_tensor_tensor(std, std, 0.5, t1,
                                   op0=ALU.mult, op1=ALU.add)
    nc.vector.reciprocal(rec, std)
    nc.vector.tensor_scalar(s_col, rec, gbt[:, 0:1], None, op0=ALU.mult)
    # t = beta - s*mean
    nc.vector.tensor_scalar(t1, mean, -1.0, None, op0=ALU.mult)
    nc.vector.scalar_tensor_tensor(
        t_col, s_col, t1, gbt[:, 1:2], op0=ALU.mult, op1=ALU.add)

    # ---- pass 2: normalize + store (f16: halves the store traffic; the
    # host widens back to f32 and permutes the [128, NB*2048] layout, whose
    # contiguous 8KB/partition runs halve the store descriptor count).
    # Normalizes split across DVE and ACT (Identity = s*y + t with
    # per-channel scale/bias APs); stores spread across all 4 HWDGE queues.
    store_q = [nc.sync, nc.scalar]
    for bp in range(NB // 2):
        ot = op_pool.tile([128, 2 * LO], mybir.dt.float16, name=f"ot{bp}", tag="ot")
        for sub in range(2):
            b = 2 * bp + sub
            dst = ot[:, sub * LO:(sub + 1) * LO]
            if sub == 1 and bp < 5:
                nc.scalar.activation(dst, y_tiles[b][:], AF.Identity,
                                     bias=t_col, scale=s_col)
            else:
                nc.vector.tensor_scalar(
                    dst, y_tiles[b][:], s_col, t_col, op0=ALU.mult, op1=ALU.add)
        store_q[bp % 2].dma_start(out[:, bp * 2 * LO:(bp + 1) * 2 * LO], ot[:])


def _prep_weights(W: np.ndarray) -> np.ndarray:
    """Host-side: binarize conv weights and pack the 8 stationary [128,128]
    lhsT matrices (4 tap-pair layouts x even/odd batch partition layouts)."""
    bw = np.sign(W).astype(np.float32)  # [CO, CI, K]
    wts = np.zeros((128, 8, 128), dtype=np.float32)
    for j in range(4):
        # even layout: rows 0:64 direct (tap 2j), rows 64:128 shifted (tap 2j+1)
        wts[0:64, j, :] = bw[:, :, 2 * j].T
        if 2 * j + 1 < K:
            wts[64:128, j, :] = bw[:, :, 2 * j + 1].T
        # odd layout: rows 0:64 shifted (tap 2j+1), rows 64:128 direct (tap 2j)
        if 2 * j + 1 < K:
            wts[0:64, 4 + j, :] = bw[:, :, 2 * j + 1].T
        wts[64:128, 4 + j, :] = bw[:, :, 2 * j].T
    return wts.reshape(128, 8 * 128).astype(ml_dtypes.float8_e4m3fn)


_NC_CACHE = {}


def kernel(x, W, prelu_w, gamma, beta):
    x = np.asarray(x)
    W = np.asarray(W)
    alpha = float(np.asarray(prelu_w).reshape(-1)[0])
    gamma = np.asarray(gamma, dtype=np.float32)
    beta = np.asarray(beta, dtype=np.float32)

    assert x.shape == (N, CI, L), x.shape
    wts = _prep_weights(W)
    gb = np.stack([gamma, beta], axis=1).astype(np.float32)

    key = alpha
    if key not in _NC_CACHE:
        _NC_CACHE[key] = _build(alpha)
    nc = _NC_CACHE[key]

    xb = x.astype(ml_dtypes.bfloat16)  # sign-preserving; halves H2D + HBM
    in_maps = []
    for c in range(N_CORES):
        shard = np.ascontiguousarray(
            xb[c * NB:(c + 1) * NB].reshape(NB * CI, L))
        in_maps.append({"xs": shard, "wts": wts, "gb": gb})

    res = run_bass_kernel_spmd(nc, in_maps, core_ids=list(range(N_CORES)))
    outs = [res.results[c]["out"].reshape(CO, NB, LO).transpose(1, 0, 2)
            for c in range(N_CORES)]
    return np.concatenate(outs, axis=0).astype(np.float32)


# revision 51
# speedup vs baseline: 1.1137x; 1.1137x over previous
"""Binarized conv1d (k=7, pad=3 with -1.0) + maxpool(2) + PReLU + BatchNorm1d
(training stats) fused Trainium2 kernel, data-parallel over batch N across 8
NeuronCores with an on-chip AllReduce for the BN batch statistics.

Contract: kernel(**inputs) takes the FULL inputs from setup_inputs() and
returns the FULL [128, 128, 2048] float32 output.

Changes over the 218us baseline:
  - ramp: pair-0's x load split into 3 column chunks on 3 different HWDGE
    queues (vector/tensor/sync) so they land in parallel; pair-0's at-copies
    ride the scalar queue so they never sit behind pair-1's 2.1MB x load;
    sign/copies/matmuls emitted per chunk (first matmul ~9us instead of 31us).
  - steady: per batch, half0 drains via the B scheme (ACT PReLU straight
    off PSUM — prelu is monotone — then a 2-port DVE f16 strided max with
    the sum(y) accum) and half1 entirely on DVE (tensor_reduce max off PSUM,
    then prelu as (m*alpha) max m, valid for 0<alpha<1). sum(y^2) runs
    mostly on ACT (Square+accum). ACT and DVE both land ~10.2us/pair; the
    PE (~8.7us/pair) keeps up. (The gpsimd/Q7 engine measured ~15ns/col on
    tensor ops — useless for any of this, it only keeps the pad memsets.)
  - output written as [128, NB*2048] f16 (contiguous 8KB/partition runs per
    pair store = half the store descriptors); host permutes to [NB,CO,LO].
  - pass-2 normalize split across DVE (tensor_scalar) and ACT (Identity with
    per-channel scale/bias APs); stores spread across all 4 HWDGE queues.
"""

import uuid

import numpy as np
import ml_dtypes
import jax
import bass_rust as _br

# The jax persistent compilation cache mis-keys bass_exec custom-call
# executables (the embedded NEFF differs while the cache key does not),
# which can hand back a stale executable and wedge the device. Disable it.
jax.config.update("jax_enable_compilation_cache", False)

import concourse.bacc as bacc
import concourse.mybir as mybir
import concourse.tile as tile
from concourse.bass_utils import run_bass_kernel_spmd

AF = mybir.ActivationFunctionType
ALU = mybir.AluOpType

N_CORES = 8
N = 128            # total batch
NB = N // N_CORES  # batches per core = 16
CI = 64            # in channels
CO = 128           # out channels
L = 4096           # input length
LP = L + 7         # padded width: cols 0-2 pad, 3..4098 data, 4099-4102 pad
LO = L // 2        # pooled output length = 2048
K = 7              # kernel taps
PAD_VAL = -1.0
EPS = 1e-5
M_GLOBAL = float(N * LO)  # BN reduction count per channel

XT_BUFS = 4
AT_BUFS = 4
# pair-0 ramp column chunk boundaries (sign/copy/matmul pipeline): chunk A
# is small so the first sign+copies+matmuls fire as early as possible
# (the whole device's HBM is saturated during the ramp — all 8 cores pull
# their pair-0 concurrently — so the first 0.13MB is what we can get early).
# A covers the t0 matmuls, B covers t1-t3, C covers half 1.
C0 = 520
C1 = 2056
# steady-state sign/copy split: halves the ACT head-of-line blocking and
# lets each pair's first matmuls start after only half the copies
SPLIT = 2056

# Engine load-balance: ACT carries sign (3.6us/pair) + 2 B-half prelus
# (1.85 each) + most of the sumsq; DVE carries 2 B-half pools (1.14) + 2
# DVE-only half drains (2.2 + 1.15) + the sumsq remainder.
SQ_ACT = 1280  # cols [LO-SQ_ACT, LO) of each batch's sumsq on ACT


def _build(alpha: float):
    nc = bacc.Bacc("TRN2", target_bir_lowering=False, debug=False,
                   num_devices=N_CORES)

    # x ships as bf16: the kernel only consumes sign(x), which the cast
    # preserves; this halves the host->device bytes, the HBM read traffic,
    # and the SBUF x-tile footprint
    xs = nc.dram_tensor("xs", [NB * CI, L], mybir.dt.bfloat16, kind="ExternalInput")
    wts = nc.dram_tensor("wts", [128, 8 * 128], mybir.dt.float8e4, kind="ExternalInput")
    gb = nc.dram_tensor("gb", [128, 2], mybir.dt.float32, kind="ExternalInput")
    out = nc.dram_tensor("out", [128, NB * LO], mybir.dt.float16, kind="ExternalOutput")

    with tile.TileContext(nc) as tc:
        with (
            tc.tile_pool(name="wp", bufs=1) as wp,
            tc.tile_pool(name="xp", bufs=XT_BUFS) as xp,
            tc.tile_pool(name="ap", bufs=AT_BUFS) as ap_pool,
            tc.tile_pool(name="pp", bufs=2, space="PSUM") as pp,
            tc.tile_pool(name="mp", bufs=4) as mp,
            tc.tile_pool(name="yp", bufs=NB) as yp,
            tc.tile_pool(name="sp", bufs=1) as sp,
            tc.tile_pool(name="qp", bufs=2) as qp,
            tc.tile_pool(name="op", bufs=4) as op_pool,
        ):
            # weights/params go via the scalar-engine HWDGE queue so the
            # ramp x chunks own the sync/vector/tensor queues from t=0
            wt = wp.tile([128, 8 * 128], mybir.dt.float8e4)
            nc.scalar.dma_start(wt[:], wts[:])
            gbt = wp.tile([128, 2], mybir.dt.float32)
            nc.scalar.dma_start(gbt[:], gb[:])

            # trigger the ACT table load during the DMA ramp, off-path
            # (the Sqrt table would otherwise load lazily right inside the
            # stats->all-reduce critical tail, costing ~1.3us on every core)
            warm = wp.tile([128, 1], mybir.dt.float32)
            nc.vector.memset(warm[:], 1.0)
            nc.scalar.activation(warm[:], warm[:], AF.Sign)
            nc.scalar.activation(warm[:], warm[:], AF.Sqrt)

            _build_pass(nc, tc, xs, out, wt, gbt, alpha,
                        xp, ap_pool, pp, mp, yp, sp, qp, op_pool)

    nc.compile()
    nc.m.name = f"bk{uuid.uuid4().hex[:10]}"
    return nc


def _build_pass(nc, tc, xs, out, wt, gbt, alpha,
                xp, ap_pool, pp, mp, yp, sp, qp, op_pool):
    # stats: cols 0:32 per-half-tile sum accums, 32:48 per-batch sumsq (DVE
    # half), 48:64 per-batch sumsq (ACT half)
    stats = sp.tile([128, 64], mybir.dt.float32, name="stats", tag="stats")

    # all-reduce buffers (persistent pool: stable addresses, single-writer
    # slots, so remote writes may land any time after the barrier)
    xbuf = sp.tile([128, 16], mybir.dt.float32, name="xbuf", tag="xbuf")
    g = sp.tile([128, 2], mybir.dt.float32, name="g", tag="g")
    rsem = nc.alloc_semaphore("ar_remote")
    lsem = nc.alloc_semaphore("ar_local")

    def _ar_descgen():
        # Each core broadcasts its [128,2] partial (sum, sumsq) into peer
        # SBUFs: for delta in 1..7, core c sends to core c^delta, landing in
        # xbuf slot delta; the XOR pairing makes every slot single-writer.
        # Descriptor generation runs here, during pass 1 while gpsimd idles;
        # the descriptors' deferred source read means only trigger_dma + the
        # 1KB transfers sit on the post-stats critical path.
        for delta in range(1, 8):
            rd = [None] * 8
            rd[delta] = (0, delta)
            nc.gpsimd.remote_dma_broadcast(
                xbuf[:, 2 * delta:2 * delta + 2], xbuf[:, 0:2],
                rsem, lsem, rdests=rd,
            )

    def _dr_rhs(at, col):
        # DoubleRow moving operand: [128, 2 k-tiles, 512] where the 2
        # contraction elements per cell are the activation at col+{0,+2}
        # (adjacent tap-pair planes = column shifts in the same at buffer).
        rhs = at[:, col:col + 512].copy()
        part = rhs.ap[0]
        rhs.ap = _br.VecI64Pair([(part[0], part[1]), (2, 2), (1, 512)])
        return rhs

    def _xt_load(bp):
        xt = xp.tile([128, LP], mybir.dt.bfloat16, name=f"xt{bp}", tag="xt")
        # the x DMA only writes 3:4099; the pad columns of the xt tile are
        # NEVER read — the binarize skips them and the at-tile pads are
        # memset directly in fp8 (see _pair_prep)
        # everything rides the sync HWDGE queue; emission order guarantees
        # each pair's latency-critical at-copies precede the NEXT pair's big
        # x transfer, so the copies (which gate the matmuls) never queue
        # behind a 6us load
        nc.sync.dma_start(xt[:, 3:L + 3], xs[bp * 128:(bp + 1) * 128, :])
        return xt

    def _xt_load0():
        # Pair 0's load is column-split across THREE pool tiles: A and B
        # FIFO on the sync queue (A lands in ~2us, its sign fires early).
        # Chunk C is NOT loaded here: it is emitted inside _prep0_chunk's
        # caller AFTER the A/B at-copies on the scalar queue, so the HWDGE
        # head-of-line blocking prioritizes the latency-critical copies over
        # C's bulk (during the ramp the device HBM is saturated by all 8
        # cores and bulk descriptors starve latency-critical ones).
        xta = xp.tile([128, LP], mybir.dt.bfloat16, name="xt0a", tag="xt")
        xtb = xp.tile([128, LP], mybir.dt.bfloat16, name="xt0b", tag="xt")
        xtc = xp.tile([128, LP], mybir.dt.bfloat16, name="xt0c", tag="xt")
        nc.sync.dma_start(xta[:, 3:C0], xs[0:128, 0:C0 - 3])
        nc.sync.dma_start(xtb[:, C0:C1], xs[0:128, C0 - 3:C1 - 3])
        return xta, xtb, xtc

    def _pair_tiles(bp):
        at0 = ap_pool.tile([128, LP], mybir.dt.float8e4,
                           name=f"at{2 * bp}", tag="at")
        at1 = ap_pool.tile([128, LP], mybir.dt.float8e4,
                           name=f"at{2 * bp + 1}", tag="at")
        return at0, at1

    def _binarize(dst, src):
        # sign(x) in fp8 on ACT (per-lane throughput: [128,*] costs the
        # same as [64,*]). The Q7/gpsimd engine measured ~15ns/col on
        # tensor_scalar — 18x too slow to take any of this work.
        nc.scalar.activation(dst, src, AF.Sign)

    def _pair_prep(bp, xt):
        # Binarize per batch-pair, split in two column chunks so the ACT
        # queue interleaves prelus between the halves and the first-half
        # copies (which gate the next pair's first matmuls) issue ~2us
        # sooner. It lands in the even batch's tile at0: rows 0:64 are b0's
        # direct half already; rows 64:128 transiently hold b1's direct
        # data, which is copied out to at1 (direct + shifted) before being
        # overwritten by b0's shifted copy. b1's matmuls run first.
        at0, at1 = _pair_tiles(bp)
        _pair_prep_chunk(at0, at1, xt, 0, SPLIT)
        return at0, at1

    def _pair_prep_chunk(at0, at1, xt, c0, c1):
        # binarize only the data columns; the pad columns are stamped
        # directly in fp8 (sign(-1) = -1) so the xt pads are never read.
        # Chunk 2 is emitted separately, AFTER the consuming pair's b0
        # half-0 drain, so the ACT prelu that frees the PSUM buffer is not
        # queued behind 3.8us of sign work (which would stall the PE).
        if c0 == 0:
            nc.vector.memset(at0[:, 0:3], -1.0)
            _binarize(at0[:, 3:c1], xt[:, 3:c1])
        elif c1 == LP:
            nc.vector.memset(at0[:, L + 3:LP], -1.0)
            _binarize(at0[:, c0:L + 3], xt[:, c0:L + 3])
        else:
            _binarize(at0[:, c0:c1], xt[:, c0:c1])
        nc.sync.dma_start(at1[64:128, c0:c1], at0[64:128, c0:c1])
        lo = max(c0 - 1, 0)
        nc.sync.dma_start(at1[0:64, lo:c1 - 1], at0[64:128, lo + 1:c1])
        nc.sync.dma_start(at0[64:128, lo:c1 - 1], at0[0:64, lo + 1:c1])

    def _prep0_chunk(at0, at1, xt, c0, c1, eng):
        # ramp: binarize+copies+shift for columns [c0, c1) only, tracking
        # the chunked x load. The copies are latency-critical (they gate the
        # matmuls) and tiny, while the device HBM is saturated by every
        # core's ramp; each chunk's copies sit at the HEAD of a HWDGE queue
        # (eng) so the queue's head-of-line blocking holds the bulk streams
        # queued behind them until the copies issue. Pad columns are stamped
        # in fp8, never read from xt.
        if c0 == 0:
            nc.vector.memset(at0[:, 0:3], -1.0)
            _binarize(at0[:, 3:c1], xt[:, 3:c1])
        elif c1 == LP:
            nc.vector.memset(at0[:, L + 3:LP], -1.0)
            _binarize(at0[:, c0:L + 3], xt[:, c0:L + 3])
        else:
            _binarize(at0[:, c0:c1], xt[:, c0:c1])
        eng.dma_start(at1[64:128, c0:c1], at0[64:128, c0:c1])
        lo = max(c0 - 1, 0)
        eng.dma_start(at1[0:64, lo:c1 - 1], at0[64:128, lo + 1:c1])
        eng.dma_start(at0[64:128, lo:c1 - 1], at0[0:64, lo + 1:c1])

    y_tiles = [None] * NB
    pending_sq = []

    def _emit_sq():
        # sum(y^2) split across both fast engines (the gpsimd/Pool engine
        # cannot run TensorScalarPtr/Square on real ISA): DVE takes the first
        # chunk (written by the earlier half0 drain, so it can start during
        # half1's ACT prelu), ACT Square-accum takes the rest.
        b, yt = pending_sq.pop(0)
        sd = LO - SQ_ACT
        sq = qp.tile([128, LO], mybir.dt.bfloat16, name=f"sq{b}", tag="sq")
        nc.scalar.activation(sq[:, sd:LO], yt[:, sd:LO],
                             AF.Square, accum_out=stats[:, 48 + b:49 + b])
        nc.vector.scalar_tensor_tensor(
            sq[:, 0:sd], yt[:, 0:sd], 1.0, yt[:, 0:sd],
            op0=ALU.mult, op1=ALU.mult,
            accum_out=stats[:, 32 + b:33 + b],
        )

    prelu_on_dve = 0.0 < alpha < 1.0

    def _drain_half(b, half, ps, yt):
        hidx = 2 * b + half
        sum_col = stats[:, hidx:hidx + 1]
        dst = yt[:, half * 1024:(half + 1) * 1024]
        if half == 0 or b == NB - 2:
            # B scheme: PReLU straight off PSUM on ACT (prelu is monotone,
            # so prelu(max)=max(prelu)), then a 2-port pooled max on DVE
            # (SBUF f16), accumulating sum(y). Only ONE operand may read
            # PSUM, so this is the cheapest drain: 1.85 ACT + 1.14 DVE.
            pt = mp.tile([128, 2048], mybir.dt.float16,
                         name=f"pt{hidx}", tag="pt")
            nc.scalar.activation(pt[:], ps[:], AF.Prelu, alpha=alpha)
            ptr = pt.rearrange("p (t two) -> p two t", two=2)
            nc.vector.scalar_tensor_tensor(
                dst, ptr[:, 0, :], 1.0, ptr[:, 1, :],
                op0=ALU.mult, op1=ALU.max,
                accum_out=sum_col,
            )
        else:
            # DVE-only drain: 1-port max off PSUM, then prelu as
            # max(alpha*m, m) (valid for 0<alpha<1) with the sum accum
            mt = mp.tile([128, 1024], mybir.dt.float16,
                         name=f"mt{hidx}", tag="mt")
            nc.vector.tensor_reduce(
                mt[:],
                ps.rearrange("p (t two) -> p t two", two=2),
                axis=mybir.AxisListType.X, op=ALU.max,
            )
            if prelu_on_dve:
                nc.vector.scalar_tensor_tensor(
                    dst, mt[:], alpha, mt[:],
                    op0=ALU.mult, op1=ALU.max,
                    accum_out=sum_col,
                )
            else:
                nc.scalar.activation(dst, mt[:], AF.Prelu, alpha=alpha,
                                     accum_out=sum_col)

    # ---- pair 0: chunked ramp ----
    xta, xtb, xtc = _xt_load0()
    at0, at1 = _pair_tiles(0)
    _prep0_chunk(at0, at1, xta, 0, C0, nc.sync)

    yt1 = yp.tile([128, LO], mybir.dt.float16, name="yt1", tag="yt")
    y_tiles[1] = yt1
    yt0 = yp.tile([128, LO], mybir.dt.float16, name="yt0", tag="yt")
    y_tiles[0] = yt0

    def _mm0(ps, ts):
        # b1 matmul group for 512-tiles ts (j0 = 4,6: odd weight layouts)
        for t in ts:
            for pair in range(2):
                j0 = 4 + 2 * pair
                w_pair = wt[:, j0 * 128:(j0 + 2) * 128].rearrange(
                    "p (e c) -> p e c", e=2)
                nc.tensor.matmul(
                    ps[:, (t % 4) * 512:(t % 4 + 1) * 512], w_pair,
                    _dr_rhs(at1, t * 512 + 4 * pair),
                    start=(pair == 0), stop=(pair == 1),
                    perf_mode=mybir.MatmulPerfMode.DoubleRow,
                )

    ps10 = pp.tile([128, 2048], mybir.dt.float32, name="ps1_0", tag="ps")
    _mm0(ps10, (0,))
    _prep0_chunk(at0, at1, xtb, C0, C1, nc.scalar)
    # chunk C's load rides the scalar queue BEHIND the chunk-B copies; the
    # queue's head-of-line blocking keeps its bulk descriptors from
    # starving the copies that gate the matmuls
    nc.scalar.dma_start(xtc[:, C1:L + 3], xs[0:128, C1 - 3:L])
    _mm0(ps10, (1, 2, 3))
    _prep0_chunk(at0, at1, xtc, C1, LP, nc.scalar)
    # pair-1's x load rides the scalar queue at the very END of the ramp
    # chain (behind the chunk-C copies): until every copy has issued, no
    # other bulk stream may touch the DMA engines
    xt_next = xp.tile([128, LP], mybir.dt.bfloat16, name="xt1", tag="xt")
    nc.scalar.dma_start(xt_next[:, 3:L + 3], xs[128:256, :])
    _drain_half(1, 0, ps10, yt1)
    ps11 = pp.tile([128, 2048], mybir.dt.float32, name="ps1_1", tag="ps")
    for pair in range(2):
        j0 = 4 + 2 * pair
        w_pair = wt[:, j0 * 128:(j0 + 2) * 128].rearrange(
            "p (e c) -> p e c", e=2)
        for t in range(4):
            nc.tensor.matmul(
                ps11[:, t * 512:(t + 1) * 512], w_pair,
                _dr_rhs(at1, 2048 + t * 512 + 4 * pair),
                start=(pair == 0), stop=(pair == 1),
                perf_mode=mybir.MatmulPerfMode.DoubleRow,
            )
    # next pair's prep between the two batches (as in steady state)
    prep_next = _pair_prep(1, xt_next)
    _pair_prep_chunk(prep_next[0], prep_next[1], xt_next, SPLIT, LP)
    _drain_half(1, 1, ps11, yt1)
    # b0: even weight layouts, at0
    for half in range(2):
        h = half * 2048
        ps = pp.tile([128, 2048], mybir.dt.float32, name=f"ps0_{half}", tag="ps")
        for pair in range(2):
            j0 = 2 * pair
            w_pair = wt[:, j0 * 128:(j0 + 2) * 128].rearrange(
                "p (e c) -> p e c", e=2)
            for t in range(4):
                nc.tensor.matmul(
                    ps[:, t * 512:(t + 1) * 512], w_pair,
                    _dr_rhs(at0, h + t * 512 + 4 * pair),
                    start=(pair == 0), stop=(pair == 1),
                    perf_mode=mybir.MatmulPerfMode.DoubleRow,
                )
        _drain_half(0, half, ps, yt0)
    pending_sq.append((1, yt1))
    pending_sq.append((0, yt0))
    while len(pending_sq) > 1:
        _emit_sq()

    # ---- pairs 1..7: steady state ----
    for bp in range(1, NB // 2):
        at0, at1 = prep_next
        # prefetch the next pair's input BEFORE this pair's remaining
        # at-copies enter the sync HWDGE queue
        if bp + 1 < NB // 2:
            xt_next = _xt_load(bp + 1)

        for sub in (1, 0):
            b = 2 * bp + sub
            at = at1 if sub == 1 else at0

            # emit the next pair's sign+copies between this pair's two
            # batches: the ACT engine starts the sign as soon as the x tile
            # lands instead of after this pair's prelus, and the PE never
            # starves waiting for at-tiles
            if sub == 0 and bp + 1 < NB // 2:
                prep_next = _pair_prep(bp + 1, xt_next)
                xt_prep = xt_next

            yt = yp.tile([128, LO], mybir.dt.float16, name=f"yt{b}", tag="yt")
            y_tiles[b] = yt

            for half in range(2):
                h = half * 2048
                ps = pp.tile([128, 2048], mybir.dt.float32,
                             name=f"ps{b}_{half}", tag="ps")
                for pair in range(2):
                    j0 = sub * 4 + 2 * pair
                    w_pair = wt[:, j0 * 128:(j0 + 2) * 128].rearrange(
                        "p (e c) -> p e c", e=2)
                    for t in range(4):
                        nc.tensor.matmul(
                            ps[:, t * 512:(t + 1) * 512],
                            w_pair,
                            _dr_rhs(at, h + t * 512 + 4 * pair),
                            start=(pair == 0), stop=(pair == 1),
                            perf_mode=mybir.MatmulPerfMode.DoubleRow,
                        )
                _drain_half(b, half, ps, yt)
                if sub == 0 and half == 0 and bp + 1 < NB // 2:
                    _pair_prep_chunk(prep_next[0], prep_next[1], xt_prep,
                                     SPLIT, LP)

            # defer this batch's sum(y^2) by one batch so it never sits
            # between a PSUM-draining pool and the PE — except in the last
            # pair, where flushing eagerly keeps the final sq off the
            # stats->all-reduce critical tail
            pending_sq.append((b, yt))
            while len(pending_sq) > (0 if bp == NB // 2 - 1 else 1):
                _emit_sq()
        if bp == 2:
            _ar_descgen()
    while pending_sq:
        _emit_sq()

    # ---- local partial stats -> remote-DMA all-reduce -> scale/shift ----
    nc.vector.tensor_reduce(xbuf[:, 0:1], stats[:, 0:32],
                            axis=mybir.AxisListType.X, op=ALU.add)
    nc.vector.tensor_reduce(xbuf[:, 1:2], stats[:, 32:64],
                            axis=mybir.AxisListType.X, op=ALU.add)

    # no_gpsimd_drain: skip the ~45us SWDGE dge_drain at block exit; the
    # kernel-tail drain picks the rings up later, off the critical path.
    with tc.tile_critical(no_gpsimd_drain=True):
        # barrier: no core fires remote writes until every peer has started
        # (prelude AllGather, normally long satisfied by now). The triggers
        # additionally inherit the deferred read of xbuf slot 0, so they wait
        # for the local stats automatically.
        nc.gpsimd.bir_kernel_barrier_wait([list(range(N_CORES))])
        nc.gpsimd.trigger_dma(count=None)
        # 7 arriving broadcasts x (16//8)=2 incs each. Staged waits with
        # marker ops so the trace shows each broadcast's arrival time
        # (distinguishes core skew from descriptor-execution serialization).
        arr = sp.tile([128, 8], mybir.dt.float32, name="arr", tag="arr")
        for k in range(1, 7):
            nc.vector.wait_ge(rsem, 2 * k)
            nc.vector.tensor_scalar(arr[:, k:k + 1], xbuf[:, 0:1], float(k),
                                    None, op0=ALU.mult)
        nc.vector.wait_ge(rsem, 14)
        nc.vector.tensor_reduce(
            g[:, 0:1], xbuf.rearrange("p (s two) -> p two s", two=2)[:, 0:1, :],
            axis=mybir.AxisListType.X, op=ALU.add)
        nc.vector.tensor_reduce(
            g[:, 1:2], xbuf.rearrange("p (s two) -> p two s", two=2)[:, 1:2, :],
            axis=mybir.AxisListType.X, op=ALU.add)

    # mean/var/scale/shift, all [128,1] f32
    v = sp.tile([128, 8], mybir.dt.float32, name="v", tag="v")
    mean, msq_eps, vareps, std, rec, t1, s_col, t_col = (
        v[:, i:i + 1] for i in range(8))
    nc.vector.tensor_scalar(mean, g[:, 0:1], 1.0 / M_GLOBAL, None, op0=ALU.mult)
    # msq_eps = mean^2 - eps
    nc.vector.tensor_scalar(msq_eps, mean, mean, EPS, op0=ALU.mult, op1=ALU.subtract)
    # vareps = ssq/M - (mean^2 - eps) = var + eps
    nc.vector.scalar_tensor_tensor(
        vareps, g[:, 1:2], 1.0 / M_GLOBAL, msq_eps,
        op0=ALU.mult, op1=ALU.subtract)
    nc.scalar.activation(std, vareps, AF.Sqrt)
    # one Newton step: std = 0.5*(std + vareps/std)
    nc.vector.reciprocal(rec, std)
    # t1 = 0.5 * vareps / std
    nc.vector.tensor_scalar(t1, rec, vareps, 0.5, op0=ALU.mult, op1=ALU.mult)
    nc.vector.scalar_tensor_tensor(std, std, 0.5, t1,
                                   op0=ALU.mult, op1=ALU.add)
    nc.vector.reciprocal(rec, std)
    nc.vector.tensor_scalar(s_col, rec, gbt[:, 0:1], None, op0=ALU.mult)
    # t = beta - s*mean
    nc.vector.tensor_scalar(t1, mean, -1.0, None, op0=ALU.mult)
    nc.vector.scalar_tensor_tensor(
        t_col, s_col, t1, gbt[:, 1:2], op0=ALU.mult, op1=ALU.add)

    # ---- pass 2: normalize + store (f16: halves the store traffic; the
    # host widens back to f32 and permutes the [128, NB*2048] layout, whose
    # contiguous 8KB/partition runs halve the store descriptor count).
    # Normalizes split across DVE and ACT (Identity = s*y + t with
    # per-channel scale/bias APs); stores spread across all 4 HWDGE queues.
    store_q = [nc.sync, nc.scalar]
    for bp in range(NB // 2):
        ot = op_pool.tile([128, 2 * LO], mybir.dt.float16, name=f"ot{bp}", tag="ot")
        for sub in range(2):
            b = 2 * bp + sub
            dst = ot[:, sub * LO:(sub + 1) * LO]
            if sub == 1 and bp < 5:
                nc.scalar.activation(dst, y_tiles[b][:], AF.Identity,
                                     bias=t_col, scale=s_col)
            else:
                nc.vector.tensor_scalar(
                    dst, y_tiles[b][:], s_col, t_col, op0=ALU.mult, op1=ALU.add)
        store_q[bp % 2].dma_start(out[:, bp * 2 * LO:(bp + 1) * 2 * LO], ot[:])


def _prep_weights(W: np.ndarray) -> np.ndarray:
    """Host-side: binarize conv weights and pack the 8 stationary [128,128]
    lhsT matrices (4 tap-pair layouts x even/odd batch partition layouts)."""
    bw = np.sign(W).astype(np.float32)  # [CO, CI, K]
    wts = np.zeros((128, 8, 128), dtype=np.float32)
    for j in range(4):
        # even layout: rows 0:64 direct (tap 2j), rows 64:128 shifted (tap 2j+1)
        wts[0:64, j, :] = bw[:, :, 2 * j].T
        if 2 * j + 1 < K:
            wts[64:128, j, :] = bw[:, :, 2 * j + 1].T
        # odd layout: rows 0:64 shifted (tap 2j+1), rows 64:128 direct (tap 2j)
        if 2 * j + 1 < K:
            wts[0:64, 4 + j, :] = bw[:, :, 2 * j + 1].T
        wts[64:128, 4 + j, :] = bw[:, :, 2 * j].T
    return wts.reshape(128, 8 * 128).astype(ml_dtypes.float8_e4m3fn)


_NC_CACHE = {}


def kernel(x, W, prelu_w, gamma, beta):
    x = np.asarray(x)
    W = np.asarray(W)
    alpha = float(np.asarray(prelu_w).reshape(-1)[0])
    gamma = np.asarray(gamma, dtype=np.float32)
    beta = np.asarray(beta, dtype=np.float32)

    assert x.shape == (N, CI, L), x.shape
    wts = _prep_weights(W)
    gb = np.stack([gamma, beta], axis=1).astype(np.float32)

    key = alpha
    if key not in _NC_CACHE:
        _NC_CACHE[key] = _build(alpha)
    nc = _NC_CACHE[key]

    xb = x.astype(ml_dtypes.bfloat16)  # sign-preserving; halves H2D + HBM
    in_maps = []
    for c in range(N_CORES):
        shard = np.ascontiguousarray(
            xb[c * NB:(c + 1) * NB].reshape(NB * CI, L))
        in_maps.append({"xs": shard, "wts": wts, "gb": gb})

    res = run_bass_kernel_spmd(nc, in_maps, core_ids=list(range(N_CORES)))
    outs = [res.results[c]["out"].reshape(CO, NB, LO).transpose(1, 0, 2)
            for c in range(N_CORES)]
    return np.concatenate(outs, axis=0).astype(np.float32)
